# revision 30
# baseline (speedup 1.0000x reference)
"""MeanStdFilter kernel for 8 Trainium2 NeuronCores (v6).

Semantics (matches the sequential-Welford reference with M=0, S=S_in, n=0):
    S1[f] = sum_b x[b, f]            (global, over all 32768 rows)
    S2[f] = sum_b x[b, f]^2
    mean  = S1 / N
    var   = (S2 - S1^2/N + S_in) / (N - 1)
    out   = (x - mean) / (sqrt(var) + 1e-5)
The input running-mean buffer M is overwritten by the first Welford step in
the reference, so it never affects the output.

Architecture (baseline 200.5us -> v5 158.7us -> this):
  - x loaded as float32r; per-chunk S1 ones-matmuls + global fp16 S2
    matmuls over Act squares, same-weight matmuls grouped in pairs so
    LDWEIGHTS alternation doesn't break the PE pipeline.
  - All [128,F] per-feature broadcasts (chunk mean, global mean) are PE
    outer-products (ones_row x row) into PSUM — no DRAM bounce, no
    queueing behind bulk loads.
  - Pre-AllReduce pass1 on DVE: y16 = x - m~_c (m~_c from PSUM), hidden
    under the load phase. fp32 math, fp16 result: output error stays
    multiplicative in |out|.
  - S1 chunk totals accumulated on the otherwise-idle GpSimd engine so
    neither PE (bank reuse anti-dep) nor DVE (busy with pass1) stalls.
  - 4KB fp16 AllReduce payload, prescaled so cols 0:1024 are the global
    mean directly. Warmup AllReduce on uninitialized scratch issued
    first pulls the one-time CC-init/skew barrier into the load phase.
  - Post-AR: delta_c = gm_psum - m~_c_psum (PE outer-products, DVE
    subtract), then per half-chunk: y -= delta_c; y *= rstd (all-fp16
    DVE 2x mode), fp16 stores (half write traffic; host upcasts). rstd
    chain runs packed [128,8] concurrently with the first pass2 pieces.
"""

import functools

import numpy as np

import concourse.bacc as bacc
import concourse.tile as tile
from concourse import mybir
from concourse.bass_utils import run_bass_kernel_spmd

NCORES = 8
B, F = 32768, 1024
ROWS = B // NCORES  # 4096 rows per core
P = 128
NT = ROWS // P  # 32 row-tiles of [128, 1024] per core
TPC = 8  # tiles per chunk
NCHUNK = NT // TPC  # 4
CROWS = P * TPC  # 1024 rows per chunk
EPS = 1e-5
FP32 = mybir.dt.float32
FP32R = mybir.dt.float32r
FP16 = mybir.dt.float16
AF = mybir.ActivationFunctionType
ALU = mybir.AluOpType


def build_kernel():
    nc = bacc.Bacc(
        "TRN2", target_bir_lowering=False, debug=False, num_devices=NCORES
    )
    x = nc.declare_dram_parameter("x", [ROWS, F], FP32R, isOutput=False)
    s_in = nc.declare_dram_parameter("S", [1, F], FP32, isOutput=False)
    out = nc.declare_dram_parameter("out", [ROWS, F], FP16, isOutput=True)

    x_t = x[:].rearrange("(n p) f -> n p f", p=P)
    out_t = out[:].rearrange("(n p) f -> n p f", p=P)
    groups = [list(range(NCORES))]

    with tile.TileContext(nc) as tc:
        with (
            tc.tile_pool(name="xst", bufs=2) as xst_pool,
            tc.tile_pool(name="ybuf", bufs=1) as ybuf,
            tc.tile_pool(name="sq", bufs=3) as sqpool,
            tc.tile_pool(name="db", bufs=2) as dbpool,
            tc.tile_pool(name="stats", bufs=1) as stats,
            tc.tile_pool(name="psum", bufs=1, space="PSUM") as psum,
            tc.tile_pool(name="dram", bufs=1, space="DRAM") as dram,
        ):
            WARMUP_AR = True  # measured: dropping this costs ~8us
            if WARMUP_AR:
                # Warmup AllReduce on garbage DRAM: pulls the one-time
                # CC-init / start-skew barrier into the load phase.
                wu_in = dram.tile([1, 8], FP32)
                wu_out = dram.tile([1, 8], FP32)
                nc.gpsimd.collective_compute(
                    "AllReduce",
                    ALU.add,
                    replica_groups=groups,
                    ins=[wu_in[:].opt()],
                    outs=[wu_out[:].opt()],
                )

            ones16 = stats.tile([P, 1], FP16)
            nc.vector.memset(ones16, 1.0)
            ones_row = stats.tile([1, P], FP16)  # outer-product weights
            nc.vector.memset(ones_row, 1.0)
            ones_r = stats.tile([P, 1], FP32R)
            nc.scalar.activation(
                ones_r[:], ones16[:], AF.Copy
            )  # fp32r needs a rounding producer

            # Resident normalized-intermediate shard (fp16), 64KB/partition.
            y16 = ybuf.tile([P, NT, F], FP16, name="y16")

            # PSUM: S1 chunk pair, S2 global pair, chunk-mean broadcast
            # pair, global-mean broadcast pair -> 4 small + 4 full banks.
            ps1 = [
                psum.tile([1, 512], FP32, tag=f"ps1_{h}", name=f"ps1_{h}")
                for h in range(2)
            ]
            ps2 = [
                psum.tile([1, 512], FP32, tag=f"ps2_{h}", name=f"ps2_{h}")
                for h in range(2)
            ]
            mcb = [
                psum.tile([P, 512], FP32, tag=f"mcb_{h}", name=f"mcb_{h}")
                for h in range(2)
            ]
            gmp = [
                psum.tile([P, 512], FP32, tag=f"gmp_{h}", name=f"gmp_{h}")
                for h in range(2)
            ]

            s_tot = stats.tile([1, F], FP32)
            m16 = [stats.tile([1, F], FP16, name=f"m16_{c}") for c in range(NCHUNK)]
            mcbs16 = [
                stats.tile([P, F], FP16, name=f"mcbs16_{c}") for c in range(NCHUNK)
            ]
            sinp = stats.tile([P, 8], FP32)
            dummy = stats.tile([1, 8], FP16)

            # ---- Phase A: stream chunks; stats on PE/Act, pass1 on DVE ----
            for c in range(NCHUNK):
                xc = xst_pool.tile([P, TPC, F], FP32R, tag="xst", name=f"x_c{c}")
                for j in range(TPC):
                    t = c * TPC + j
                    xt = xc[:, j, :]
                    nc.sync.dma_start(out=xt, in_=x_t[t])
                    sq = sqpool.tile([P, F], FP16, tag="sq")
                    nc.scalar.activation(sq, xt.bitcast(FP32), AF.Square)
                    for h in range(2):
                        nc.tensor.matmul(
                            ps1[h][:],
                            lhsT=ones_r[:],
                            rhs=xt[:, h * 512 : (h + 1) * 512],
                            start=(j == 0),
                            stop=(j == TPC - 1),
                        )
                    for h in range(2):
                        nc.tensor.matmul(
                            ps2[h][:],
                            lhsT=ones16[:],
                            rhs=sq[:, h * 512 : (h + 1) * 512],
                            start=(t == 0),
                            stop=(t == NT - 1),
                        )
                # Chunk stats: fp16 chunk mean m~_c (Act), running S1 total
                # (GpSimd, straight from PSUM — keeps PE/DVE unblocked),
                # then broadcast m~_c to 128 partitions via PE outer-product.
                for h in range(2):
                    hs = slice(h * 512, (h + 1) * 512)
                    nc.scalar.activation(
                        m16[c][:, hs], ps1[h][:], AF.Copy, scale=1.0 / CROWS
                    )
                # GpSimd can't read PSUM: accumulate the sum of fp16 chunk
                # means instead (costs ~2e-6 absolute on the global mean);
                # the CROWS/B scale is folded into the AR staging copy.
                if c == 0:
                    nc.gpsimd.tensor_copy(s_tot[:], m16[c])
                else:
                    nc.gpsimd.tensor_tensor(s_tot[:], s_tot, m16[c], ALU.add)
                for h in range(2):
                    nc.tensor.matmul(
                        mcb[h][:],
                        lhsT=ones_row[:],
                        rhs=m16[c][:, h * 512 : (h + 1) * 512],
                        start=True,
                        stop=True,
                    )
                # pass1: y16 = x - m~_c  (fp32 math, fp16 result), per half.
                for h in range(2):
                    hs = slice(h * 512, (h + 1) * 512)
                    nc.vector.tensor_tensor(
                        y16[:, c * TPC : (c + 1) * TPC, hs],
                        xc[:, :, hs].bitcast(FP32),
                        mcb[h][:, None, :].to_broadcast([P, TPC, 512]),
                        ALU.subtract,
                    )
                    # Keep an SBUF fp16 copy of the m~_c broadcast for the
                    # post-AR delta pass (PSUM banks get recycled).
                    nc.scalar.copy(mcbs16[c][:, hs], mcb[h][:])
                if c == 0:
                    # Deferred setup (after chunk 0 so x DMAs lead the
                    # queues; Sqrt table preload rides the Act slack).
                    nc.sync.dma_start(
                        out=sinp[:],
                        in_=s_in[:].rearrange("a (p j) -> a p j", p=P, j=8),
                    )
                    nc.scalar.activation(dummy, sinp[0:1, :], AF.Sqrt)

            # ---- AllReduce: [mean | S2/(N-1)] in fp16, 4KB ----
            cc_stage = stats.tile([1, 2 * F], FP16)
            nc.scalar.activation(
                cc_stage[:, 0:F], s_tot[:], AF.Copy, scale=float(CROWS) / B
            )
            for h in range(2):
                nc.scalar.activation(
                    cc_stage[:, F + h * 512 : F + (h + 1) * 512],
                    ps2[h][:],
                    AF.Copy,
                    scale=1.0 / (B - 1),
                )
            cc_in = dram.tile([1, 2 * F], FP16)
            cc_out = dram.tile([1, 2 * F], FP16)
            nc.sync.dma_start(out=cc_in[:], in_=cc_stage[:])
            nc.gpsimd.collective_compute(
                "AllReduce",
                ALU.add,
                replica_groups=groups,
                ins=[cc_in[:].opt()],
                outs=[cc_out[:].opt()],
            )

            # Global mean row -> PE outer-product broadcast -> SBUF fp16
            # (so the delta subtracts run in the DVE 2x mode).
            gm_row = stats.tile([1, F], FP16)
            nc.sync.dma_start(out=gm_row[:], in_=cc_out[:, 0:F])
            gms16 = stats.tile([P, F], FP16)
            for h in range(2):
                hs = slice(h * 512, (h + 1) * 512)
                nc.tensor.matmul(
                    gmp[h][:],
                    lhsT=ones_row[:],
                    rhs=gm_row[:, hs],
                    start=True,
                    stop=True,
                )
                nc.scalar.copy(gms16[:, hs], gmp[h][:])

            # ---- rstd chain, packed [128, 8] (f = p*8 + j), fp32 work ----
            s12p = stats.tile([P, 2, 8], FP16)
            nc.sync.dma_start(
                out=s12p[:],
                in_=cc_out[:].rearrange("a (h p j) -> a p h j", h=2, p=P, j=8),
            )
            a1 = s12p[:, 0, :]  # global mean (fp16)
            a2 = s12p[:, 1, :]  # sum(x^2)/(N-1) (fp16)
            finw = stats.tile([P, 32], FP32)
            w1, w2, w3, w4 = (finw[:, 8 * i : 8 * (i + 1)] for i in range(4))
            nc.vector.tensor_tensor(w1, a1, a1, ALU.mult)  # mean^2
            # var = a2 - mean^2 * N/(N-1) + S_in/(N-1)
            nc.vector.scalar_tensor_tensor(
                w2, w1, -float(B) / (B - 1), a2, ALU.mult, ALU.add
            )
            nc.vector.scalar_tensor_tensor(
                w2, sinp[:], 1.0 / (B - 1), w2, ALU.mult, ALU.add
            )
            nc.scalar.activation(w3, w2, AF.Sqrt)
            nc.scalar.activation(w4, w3, AF.Copy, bias=EPS)
            rinv = stats.tile([P, 8], FP32)
            nc.vector.reciprocal(rinv, w4)
            r16p = stats.tile([P, 8], FP16)
            nc.scalar.copy(r16p[:], rinv[:])
            rd = dram.tile([1, F], FP16)
            nc.sync.dma_start(
                out=rd[:].rearrange("a (p j) -> a p j", p=P, j=8), in_=r16p[:]
            )
            rb16 = stats.tile([P, F], FP16)
            nc.sync.dma_start(out=rb16[:], in_=rd[:].to_broadcast([P, F]))

            # ---- Phase C: a-pieces (y -= delta_c) with b-pieces (y *= rstd,
            # store) lagged one chunk behind — late enough that the rstd
            # broadcast never stalls the in-order DVE queue, early enough
            # that the fp16 stores spread over the whole phase.
            HC = TPC // 2

            def emit_a(c):
                # delta_c = global_mean - m~_c broadcast (all fp16 SBUF, 2x)
                db16 = dbpool.tile([P, F], FP16, tag="db", name=f"db_{c}")
                nc.vector.tensor_tensor(db16[:], gms16, mcbs16[c], ALU.subtract)
                db = db16[:, None, :].to_broadcast([P, HC, F])
                for p in range(2):
                    t0 = c * TPC + p * HC
                    ysl = y16[:, t0 : t0 + HC, :]
                    nc.vector.tensor_tensor(ysl, ysl, db, ALU.subtract)

            def emit_b(c):
                # 2-tile pieces: finer store granularity shrinks the final
                # store tail after the last DVE op.
                rb = rb16[:, None, :].to_broadcast([P, 2, F])
                for p in range(TPC // 2):
                    t0 = c * TPC + p * 2
                    ysl = y16[:, t0 : t0 + 2, :]
                    nc.vector.tensor_tensor(ysl, ysl, rb, ALU.mult)
                    for j in range(2):
                        t = t0 + j
                        nc.sync.dma_start(out=out_t[t], in_=y16[:, t, :])

            emit_a(0)
            for c in range(1, NCHUNK):
                emit_a(c)
                emit_b(c - 1)
            emit_b(NCHUNK - 1)

    nc.finalize()
    return nc


@functools.cache
def _get_nc():
    return build_kernel()


def kernel(x, M, S, _trace=False, _trace_kwargs=None):
    del M  # overwritten by the first Welford step in the reference
    x = np.ascontiguousarray(x, dtype=np.float32)
    S = np.ascontiguousarray(S, dtype=np.float32).reshape(1, F)
    nc = _get_nc()
    in_maps = [
        {"x": x[i * ROWS : (i + 1) * ROWS], "S": S} for i in range(NCORES)
    ]
    res = run_bass_kernel_spmd(
        nc,
        in_maps,
        core_ids=list(range(NCORES)),
        trace=_trace,
        **(_trace_kwargs or {}),
    )
    out = np.concatenate(
        [res.results[i]["out"] for i in range(NCORES)], axis=0
    ).astype(np.float32)
    if _trace:
        return out, res
    return out


# revision 31
# speedup vs baseline: 1.0360x; 1.0360x over previous
"""MeanStdFilter kernel for 8 Trainium2 NeuronCores (v6).

Semantics (matches the sequential-Welford reference with M=0, S=S_in, n=0):
    S1[f] = sum_b x[b, f]            (global, over all 32768 rows)
    S2[f] = sum_b x[b, f]^2
    mean  = S1 / N
    var   = (S2 - S1^2/N + S_in) / (N - 1)
    out   = (x - mean) / (sqrt(var) + 1e-5)
The input running-mean buffer M is overwritten by the first Welford step in
the reference, so it never affects the output.

Architecture (baseline 200.5us -> v5 158.7us -> this):
  - x loaded as float32r; per-chunk S1 ones-matmuls + global fp16 S2
    matmuls over Act squares, same-weight matmuls grouped in pairs so
    LDWEIGHTS alternation doesn't break the PE pipeline.
  - All [128,F] per-feature broadcasts (chunk mean, global mean) are PE
    outer-products (ones_row x row) into PSUM — no DRAM bounce, no
    queueing behind bulk loads.
  - Pre-AllReduce pass1 on DVE: y16 = x - m~_c (m~_c from PSUM), hidden
    under the load phase. fp32 math, fp16 result: output error stays
    multiplicative in |out|.
  - S1 chunk totals accumulated on the otherwise-idle GpSimd engine so
    neither PE (bank reuse anti-dep) nor DVE (busy with pass1) stalls.
  - 4KB fp16 AllReduce payload, prescaled so cols 0:1024 are the global
    mean directly. Warmup AllReduce on uninitialized scratch issued
    first pulls the one-time CC-init/skew barrier into the load phase.
  - Post-AR: delta_c = gm_psum - m~_c_psum (PE outer-products, DVE
    subtract), then per half-chunk: y -= delta_c; y *= rstd (all-fp16
    DVE 2x mode), fp16 stores (half write traffic; host upcasts). rstd
    chain runs packed [128,8] concurrently with the first pass2 pieces.
"""

import functools

import numpy as np

import concourse.bacc as bacc
import concourse.tile as tile
from concourse import mybir
from concourse.bass_utils import run_bass_kernel_spmd

NCORES = 8
B, F = 32768, 1024
ROWS = B // NCORES  # 4096 rows per core
P = 128
NT = ROWS // P  # 32 row-tiles of [128, 1024] per core
TPC = 8  # tiles per chunk
NCHUNK = NT // TPC  # 4
CROWS = P * TPC  # 1024 rows per chunk
EPS = 1e-5
FP32 = mybir.dt.float32
FP32R = mybir.dt.float32r
FP16 = mybir.dt.float16
AF = mybir.ActivationFunctionType
ALU = mybir.AluOpType


def build_kernel():
    nc = bacc.Bacc(
        "TRN2", target_bir_lowering=False, debug=False, num_devices=NCORES
    )
    x = nc.declare_dram_parameter("x", [ROWS, F], FP32R, isOutput=False)
    s_in = nc.declare_dram_parameter("S", [1, F], FP32, isOutput=False)
    out = nc.declare_dram_parameter("out", [ROWS, F], FP16, isOutput=True)

    x_t = x[:].rearrange("(n p) f -> n p f", p=P)
    out_t = out[:].rearrange("(n p) f -> n p f", p=P)
    groups = [list(range(NCORES))]

    with tile.TileContext(nc) as tc:
        with (
            tc.tile_pool(name="xst", bufs=3) as xst_pool,
            tc.tile_pool(name="ybuf", bufs=1) as ybuf,
            tc.tile_pool(name="sq", bufs=3) as sqpool,
            tc.tile_pool(name="db", bufs=2) as dbpool,
            tc.tile_pool(name="stats", bufs=1) as stats,
            tc.tile_pool(name="psum", bufs=1, space="PSUM") as psum,
            tc.tile_pool(name="dram", bufs=1, space="DRAM") as dram,
        ):
            WARMUP_AR = True  # measured: dropping this costs ~8us
            if WARMUP_AR:
                # Warmup AllReduce on garbage DRAM: pulls the one-time
                # CC-init / start-skew barrier into the load phase.
                wu_in = dram.tile([1, 8], FP32)
                wu_out = dram.tile([1, 8], FP32)
                nc.gpsimd.collective_compute(
                    "AllReduce",
                    ALU.add,
                    replica_groups=groups,
                    ins=[wu_in[:].opt()],
                    outs=[wu_out[:].opt()],
                )

            ones16 = stats.tile([P, 1], FP16)
            nc.vector.memset(ones16, 1.0)
            ones_row = stats.tile([1, P], FP16)  # outer-product weights
            nc.vector.memset(ones_row, 1.0)
            ones_r = stats.tile([P, 1], FP32R)
            nc.scalar.activation(
                ones_r[:], ones16[:], AF.Copy
            )  # fp32r needs a rounding producer

            # Resident normalized-intermediate shard (fp16), 64KB/partition.
            y16 = ybuf.tile([P, NT, F], FP16, name="y16")

            # PSUM: S1 chunk pair, S2 global pair, chunk-mean broadcast
            # pair, global-mean broadcast pair -> 4 small + 4 full banks.
            ps1 = [
                psum.tile([1, 512], FP32, tag=f"ps1_{h}", name=f"ps1_{h}")
                for h in range(2)
            ]
            ps2 = [
                psum.tile([1, 512], FP32, tag=f"ps2_{h}", name=f"ps2_{h}")
                for h in range(2)
            ]
            mcb = [
                psum.tile([P, 512], FP32, tag=f"mcb_{h}", name=f"mcb_{h}")
                for h in range(2)
            ]
            gmp = [
                psum.tile([P, 512], FP32, tag=f"gmp_{h}", name=f"gmp_{h}")
                for h in range(2)
            ]

            s_tot = stats.tile([1, F], FP32)
            m16 = [stats.tile([1, F], FP16, name=f"m16_{c}") for c in range(NCHUNK)]
            mcbs16 = [
                stats.tile([P, F], FP16, name=f"mcbs16_{c}") for c in range(NCHUNK)
            ]
            sinp = stats.tile([P, 8], FP32)
            dummy = stats.tile([1, 8], FP16)

            # ---- Phase A: stream chunks; stats on PE/Act, pass1 on DVE ----
            for c in range(NCHUNK):
                xc = xst_pool.tile([P, TPC, F], FP32R, tag="xst", name=f"x_c{c}")
                for j in range(TPC):
                    t = c * TPC + j
                    xt = xc[:, j, :]
                    nc.sync.dma_start(out=xt, in_=x_t[t])
                    sq = sqpool.tile([P, F], FP16, tag="sq")
                    nc.scalar.activation(sq, xt.bitcast(FP32), AF.Square)
                    for h in range(2):
                        nc.tensor.matmul(
                            ps1[h][:],
                            lhsT=ones_r[:],
                            rhs=xt[:, h * 512 : (h + 1) * 512],
                            start=(j == 0),
                            stop=(j == TPC - 1),
                        )
                    for h in range(2):
                        nc.tensor.matmul(
                            ps2[h][:],
                            lhsT=ones16[:],
                            rhs=sq[:, h * 512 : (h + 1) * 512],
                            start=(t == 0),
                            stop=(t == NT - 1),
                        )
                # Chunk stats: fp16 chunk mean m~_c (Act), running S1 total
                # (GpSimd, straight from PSUM — keeps PE/DVE unblocked),
                # then broadcast m~_c to 128 partitions via PE outer-product.
                for h in range(2):
                    hs = slice(h * 512, (h + 1) * 512)
                    nc.scalar.activation(
                        m16[c][:, hs], ps1[h][:], AF.Copy, scale=1.0 / CROWS
                    )
                # GpSimd can't read PSUM: accumulate the sum of fp16 chunk
                # means instead (costs ~2e-6 absolute on the global mean);
                # the CROWS/B scale is folded into the AR staging copy.
                if c == 0:
                    nc.gpsimd.tensor_copy(s_tot[:], m16[c])
                else:
                    nc.gpsimd.tensor_tensor(s_tot[:], s_tot, m16[c], ALU.add)
                for h in range(2):
                    nc.tensor.matmul(
                        mcb[h][:],
                        lhsT=ones_row[:],
                        rhs=m16[c][:, h * 512 : (h + 1) * 512],
                        start=True,
                        stop=True,
                    )
                # pass1: y16 = x - m~_c  (fp32 math, fp16 result), per half.
                for h in range(2):
                    hs = slice(h * 512, (h + 1) * 512)
                    nc.vector.tensor_tensor(
                        y16[:, c * TPC : (c + 1) * TPC, hs],
                        xc[:, :, hs].bitcast(FP32),
                        mcb[h][:, None, :].to_broadcast([P, TPC, 512]),
                        ALU.subtract,
                    )
                    # Keep an SBUF fp16 copy of the m~_c broadcast for the
                    # post-AR delta pass (PSUM banks get recycled).
                    nc.scalar.copy(mcbs16[c][:, hs], mcb[h][:])
                if c == 0:
                    # Deferred setup (after chunk 0 so x DMAs lead the
                    # queues; Sqrt table preload rides the Act slack).
                    nc.sync.dma_start(
                        out=sinp[:],
                        in_=s_in[:].rearrange("a (p j) -> a p j", p=P, j=8),
                    )
                    nc.scalar.activation(dummy, sinp[0:1, :], AF.Sqrt)

            # ---- AllReduce: [mean | S2/(N-1)] in fp16, 4KB ----
            cc_stage = stats.tile([1, 2 * F], FP16)
            nc.scalar.activation(
                cc_stage[:, 0:F], s_tot[:], AF.Copy, scale=float(CROWS) / B
            )
            for h in range(2):
                nc.scalar.activation(
                    cc_stage[:, F + h * 512 : F + (h + 1) * 512],
                    ps2[h][:],
                    AF.Copy,
                    scale=1.0 / (B - 1),
                )
            cc_in = dram.tile([1, 2 * F], FP16)
            cc_out = dram.tile([1, 2 * F], FP16)
            nc.sync.dma_start(out=cc_in[:], in_=cc_stage[:])
            nc.gpsimd.collective_compute(
                "AllReduce",
                ALU.add,
                replica_groups=groups,
                ins=[cc_in[:].opt()],
                outs=[cc_out[:].opt()],
            )

            # Global mean row -> PE outer-product broadcast -> SBUF fp16
            # (so the delta subtracts run in the DVE 2x mode).
            gm_row = stats.tile([1, F], FP16)
            nc.sync.dma_start(out=gm_row[:], in_=cc_out[:, 0:F])
            gms16 = stats.tile([P, F], FP16)
            for h in range(2):
                hs = slice(h * 512, (h + 1) * 512)
                nc.tensor.matmul(
                    gmp[h][:],
                    lhsT=ones_row[:],
                    rhs=gm_row[:, hs],
                    start=True,
                    stop=True,
                )
                nc.scalar.copy(gms16[:, hs], gmp[h][:])

            # ---- rstd chain, packed [128, 8] (f = p*8 + j), fp32 work ----
            s12p = stats.tile([P, 2, 8], FP16)
            nc.sync.dma_start(
                out=s12p[:],
                in_=cc_out[:].rearrange("a (h p j) -> a p h j", h=2, p=P, j=8),
            )
            a1 = s12p[:, 0, :]  # global mean (fp16)
            a2 = s12p[:, 1, :]  # sum(x^2)/(N-1) (fp16)
            finw = stats.tile([P, 32], FP32)
            w1, w2, w3, w4 = (finw[:, 8 * i : 8 * (i + 1)] for i in range(4))
            nc.vector.tensor_tensor(w1, a1, a1, ALU.mult)  # mean^2
            # var = a2 - mean^2 * N/(N-1) + S_in/(N-1)
            nc.vector.scalar_tensor_tensor(
                w2, w1, -float(B) / (B - 1), a2, ALU.mult, ALU.add
            )
            nc.vector.scalar_tensor_tensor(
                w2, sinp[:], 1.0 / (B - 1), w2, ALU.mult, ALU.add
            )
            nc.scalar.activation(w3, w2, AF.Sqrt)
            nc.scalar.activation(w4, w3, AF.Copy, bias=EPS)
            rinv = stats.tile([P, 8], FP32)
            nc.vector.reciprocal(rinv, w4)
            r16p = stats.tile([P, 8], FP16)
            nc.scalar.copy(r16p[:], rinv[:])
            rd = dram.tile([1, F], FP16)
            nc.sync.dma_start(
                out=rd[:].rearrange("a (p j) -> a p j", p=P, j=8), in_=r16p[:]
            )
            rb16 = stats.tile([P, F], FP16)
            nc.sync.dma_start(out=rb16[:], in_=rd[:].to_broadcast([P, F]))

            # ---- Phase C: a-pieces (y -= delta_c) with b-pieces (y *= rstd,
            # store) lagged one chunk behind — late enough that the rstd
            # broadcast never stalls the in-order DVE queue, early enough
            # that the fp16 stores spread over the whole phase.
            HC = TPC // 2

            def emit_a(c):
                # delta_c = global_mean - m~_c broadcast (all fp16 SBUF, 2x)
                db16 = dbpool.tile([P, F], FP16, tag="db", name=f"db_{c}")
                nc.vector.tensor_tensor(db16[:], gms16, mcbs16[c], ALU.subtract)
                db = db16[:, None, :].to_broadcast([P, HC, F])
                for p in range(2):
                    t0 = c * TPC + p * HC
                    ysl = y16[:, t0 : t0 + HC, :]
                    nc.vector.tensor_tensor(ysl, ysl, db, ALU.subtract)

            def emit_b(c):
                # 2-tile pieces: finer store granularity shrinks the final
                # store tail after the last DVE op.
                rb = rb16[:, None, :].to_broadcast([P, 2, F])
                for p in range(TPC // 2):
                    t0 = c * TPC + p * 2
                    ysl = y16[:, t0 : t0 + 2, :]
                    nc.vector.tensor_tensor(ysl, ysl, rb, ALU.mult)
                    for j in range(2):
                        t = t0 + j
                        nc.sync.dma_start(out=out_t[t], in_=y16[:, t, :])

            emit_a(0)
            for c in range(1, NCHUNK):
                emit_a(c)
                emit_b(c - 1)
            emit_b(NCHUNK - 1)

    nc.finalize()
    return nc


@functools.cache
def _get_nc():
    return build_kernel()


def kernel(x, M, S, _trace=False, _trace_kwargs=None):
    del M  # overwritten by the first Welford step in the reference
    x = np.ascontiguousarray(x, dtype=np.float32)
    S = np.ascontiguousarray(S, dtype=np.float32).reshape(1, F)
    nc = _get_nc()
    in_maps = [
        {"x": x[i * ROWS : (i + 1) * ROWS], "S": S} for i in range(NCORES)
    ]
    res = run_bass_kernel_spmd(
        nc,
        in_maps,
        core_ids=list(range(NCORES)),
        trace=_trace,
        **(_trace_kwargs or {}),
    )
    out = np.concatenate(
        [res.results[i]["out"] for i in range(NCORES)], axis=0
    ).astype(np.float32)
    if _trace:
        return out, res
    return out
